# revision 5
# baseline (speedup 1.0000x reference)
"""Multi-head self-attention with RoPE — 8-core SPMD Bass kernel for TRN2.

Problem: nn_MultiHeadSelfAttention (b=2, s=2048, d=1024, h=16, hd=64),
y = softmax(mask(RoPE(xWq^T) RoPE(xWk^T)^T / 8)) (xWv^T) Wo^T.

Sharding (tensor/head parallel): heads 2i, 2i+1 -> core i. Each core
computes Q/K/V projections for its 2 heads over the full sequence (both
batches), applies RoPE, computes attention, AllGathers the per-head
attention outputs (transposed [d, seq] layout, fp16), and computes a
128-column slice of the output projection. The host assembles the 8
column slices into the full output.

Kernel layout/engineering notes:
 - All matmul-adjacent tensors are dtype float32r (fp32 bytes, TF32-like
   matmul at full PE rate for moving dim >= 256).
 - Activations are kept transposed ([d, seq]) end-to-end so every matmul
   has its contraction on the partition axis with N=512 moving columns;
   no transposes are needed except V (32 cheap PE-transposes).
 - hd components of Q/K are host-deinterleaved (evens then odds) so the
   RoPE partner lives at a fixed partition offset; the partner shuffle is
   one PE matmul against a constant +-1 permutation matrix, and RoPE is
   3 DVE ops against host-precomputed cos/sin tables. The 1/sqrt(hd)
   scale is folded into Wq on the host.
 - Scores are computed transposed (ST[k, q]) so the key-padding mask is
   a per-partition ACT bias on the exp activation (exp(score - 30) for
   masked keys ~ 0), and the softmax denominators ride the PV matmul as
   a 65th all-ones column of V (row 64 of the PV output = row sums).
   Normalization is deferred to the [64, 512] attention outputs.
 - The two heads' QK^T matmuls have K=64 and sit at partition bases 0/64,
   so the PE row-groups run them concurrently.
 - Batches are software-pipelined in a wavefront: attention chunks are
   emitted as soon as their Q/K column blocks exist, so the ACT engine
   (exp is the 109us/core floor) starts ~20us in and runs ~95% duty.
   AllGather(b0) overlaps batch-1 compute; only AG(b1) + its out-proj
   remain exposed in the tail.
"""

from contextlib import ExitStack

import numpy as np

import concourse.bacc as bacc_mod
import concourse.tile as tile
from concourse import mybir
from concourse.bass_utils import run_bass_kernel_spmd

F32 = mybir.dt.float32
F32R = mybir.dt.float32r
F16 = mybir.dt.float16
EXP = mybir.ActivationFunctionType.Exp

B = 2
S = 2048
D = 1024
H = 16
HD = 64
NCORES = 8
HPC = H // NCORES          # 2 heads per core
S2 = B * S                 # 4096
NCB = S2 // 512            # 8 column blocks of 512
NCBB = S // 512            # 4 column blocks per batch
NKT = S // 128             # 16 key tiles per batch
NQB = S // 512             # 4 query blocks per batch
DKT = D // 128             # 8 contraction tiles for d=1024
MASK_NEG = -30.0


def build_kernel(repeats: int = 1, collectives: bool = True):
    nc = bacc_mod.Bacc("TRN2", target_bir_lowering=False, debug=False,
                       num_devices=NCORES)

    xT = nc.declare_dram_parameter("xT", [D, S2], F32R, isOutput=False)
    wq = nc.declare_dram_parameter("wq", [D, 128], F32R, isOutput=False)
    wk = nc.declare_dram_parameter("wk", [D, 128], F32R, isOutput=False)
    wv = nc.declare_dram_parameter("wv", [D, 128], F32R, isOutput=False)
    wo = nc.declare_dram_parameter("wo", [D, 128], F16, isOutput=False)
    cosT = nc.declare_dram_parameter("cosT", [128, S2], F32R, isOutput=False)
    sinT = nc.declare_dram_parameter("sinT", [128, S2], F32R, isOutput=False)
    perm = nc.declare_dram_parameter("perm", [128, 128], F32R, isOutput=False)
    ident = nc.declare_dram_parameter("ident", [128, 128], F32R, isOutput=False)
    maskb = nc.declare_dram_parameter("maskb", [128, B * NKT], F32,
                                      isOutput=False)
    onesc = nc.declare_dram_parameter("onesc", [128, B * HPC * NKT], F32R,
                                      isOutput=False)
    out = nc.declare_dram_parameter("out", [128, S2], F32, isOutput=True)

    xT_t = xT.rearrange("(kt p) c -> p kt c", p=128)
    w_t = {n: w.rearrange("(kt p) m -> p kt m", p=128)
           for n, w in (("wq", wq), ("wk", wk), ("wv", wv), ("wo", wo))}

    with tile.TileContext(nc) as tc:
        for _ in range(repeats):
            _emit_body(nc, tc, xT_t, w_t, cosT, sinT, perm, ident, maskb,
                       onesc, out, collectives)
    nc.compile()
    return nc


def _emit_body(nc, tc, xT_t, w_t, cosT, sinT, perm, ident, maskb, onesc, out,
               collectives=True):
    with ExitStack() as body:
        consts = body.enter_context(tc.tile_pool(name="consts", bufs=1))
        w_sb = {}
        for n in ("wq", "wk", "wv"):
            w_sb[n] = consts.tile([128, DKT, 128], F32R, name=f"{n}_sb")
            nc.sync.dma_start(out=w_sb[n], in_=w_t[n])
        w_sb["wo"] = consts.tile([128, DKT, 128], F16, name="wo_sb")
        perm_sb = consts.tile([128, 128], F32R)
        nc.sync.dma_start(out=perm_sb, in_=perm[:, :])
        ident_sb = consts.tile([128, 128], F32R)
        nc.sync.dma_start(out=ident_sb, in_=ident[:, :])
        maskb_sb = consts.tile([128, B * NKT], F32)
        nc.sync.dma_start(out=maskb_sb, in_=maskb[:, :])

        # persistent activations (transposed layouts)
        acts = body.enter_context(tc.tile_pool(name="acts", bufs=1))
        qrot = acts.tile([128, NCB, 512], F32R)
        krot = acts.tile([128, NCB, 512], F32R)
        v_all = acts.tile([128, B * HPC, NKT, 65], F32R)
        nc.sync.dma_start(out=v_all[:, :, :, 64],
                          in_=onesc.rearrange("p (a k) -> p a k", a=B * HPC))

        # pools with whole-kernel lifetime; PSUM banks are statically
        # partitioned (proj 2 + perm/transpose 1 + scores 3 + pv 1 + out 1)
        st_ps = body.enter_context(
            tc.tile_pool(name="st_ps", bufs=3, space="PSUM"))
        o_ps = body.enter_context(
            tc.tile_pool(name="o_ps", bufs=1, space="PSUM"))
        fo_ps = body.enter_context(
            tc.tile_pool(name="fo_ps", bufs=1, space="PSUM"))
        ppool = body.enter_context(tc.tile_pool(name="ppool", bufs=6))
        npool = body.enter_context(tc.tile_pool(name="npool", bufs=2))
        uaccp = body.enter_context(tc.tile_pool(name="uaccp", bufs=8))
        u16p = body.enter_context(tc.tile_pool(name="u16p", bufs=8))
        upool = body.enter_context(tc.tile_pool(name="upool", bufs=8))
        opool = body.enter_context(tc.tile_pool(name="opool", bufs=2))
        dram = body.enter_context(
            tc.tile_pool(name="dram", bufs=1, space="DRAM"))

        cc_out = {}
        u_acc = {}
        u_16 = {}

        def emit_B_cb(b, c):
            """Q/K/V projections + RoPE for column block c of batch b."""
            cb = b * NCBB + c
            xsb = xpool.tile([128, DKT, 512], F32R, tag="xsb", name="xsb")
            nc.sync.dma_start(out=xsb[:, 0:4, :],
                              in_=xT_t[:, 0:4, cb * 512:(cb + 1) * 512])
            nc.sync.dma_start(out=xsb[:, 4:8, :],
                              in_=xT_t[:, 4:8, cb * 512:(cb + 1) * 512])
            cos_cb = cpool.tile([128, 512], F32R, tag="cos", name="cos_cb")
            nc.sync.dma_start(out=cos_cb, in_=cosT[:, cb * 512:(cb + 1) * 512])
            sin_cb = cpool.tile([128, 512], F32R, tag="sin", name="sin_cb")
            nc.sync.dma_start(out=sin_cb, in_=sinT[:, cb * 512:(cb + 1) * 512])
            for name, dst, rope in (("wv", vt[b], False),
                                    ("wq", qrot, True),
                                    ("wk", krot, True)):
                pr = proj_ps.tile([128, 512], F32, tag="proj", name="pr")
                for kt in range(DKT):
                    nc.tensor.matmul(pr, w_sb[name][:, kt, :], xsb[:, kt, :],
                                     start=(kt == 0), stop=(kt == DKT - 1))
                if not rope:
                    nc.vector.tensor_copy(dst[:, c, :], pr)
                    continue
                raw = tmp.tile([128, 512], F32R, tag="raw", name="raw")
                nc.vector.tensor_copy(raw, pr)
                pp = proj_ps.tile([128, 512], F32, tag="pp", name="pp", bufs=1)
                nc.tensor.matmul(pp, perm_sb, raw, start=True, stop=True)
                tcos = tmp.tile([128, 512], F32R, tag="tcos", name="tcos")
                nc.vector.tensor_mul(tcos, raw, cos_cb)
                tsin = tmp.tile([128, 512], F32R, tag="tsin", name="tsin")
                nc.vector.tensor_mul(tsin, pp, sin_cb)
                nc.vector.tensor_add(dst[:, cb, :], tcos, tsin)
            # transpose this block's V into [k, hd] layout (4 key tiles)
            for kt in range(4 * c, 4 * c + 4):
                off = (kt % 4) * 128
                tp = proj_ps.tile([128, 128], F32R, tag="pp", name="tp",
                                  bufs=1)
                nc.tensor.transpose(tp, vt[b][:, c, off:off + 128], ident_sb)
                for ln in range(HPC):
                    nc.vector.tensor_copy(
                        v_all[:, b * HPC + ln, kt, 0:64],
                        tp[:, ln * 64:(ln + 1) * 64])

        def emit_C_chunk(b, qb, j):
            """Attention chunk: query block qb vs key tiles 4j..4j+3."""
            cb_q = b * NQB + qb
            for ln in range(HPC):
                if j == 0:
                    u_acc[(b, qb, ln)] = uaccp.tile(
                        [65, 512], F32, tag="uacc", name=f"ua{qb}{ln}")
                oc = o_ps.tile([65, 512], F32, tag="oc", name=f"oc{ln}")
                for kt in range(4 * j, 4 * j + 4):
                    cb_k, off = divmod(b * S + kt * 128, 512)
                    mb = maskb_sb[:, (b * NKT + kt):(b * NKT + kt) + 1]
                    st = st_ps.tile([128, 512], F32, tag="st", name=f"st{ln}")
                    nc.tensor.matmul(
                        st,
                        krot[ln * 64:(ln + 1) * 64, cb_k, off:off + 128],
                        qrot[ln * 64:(ln + 1) * 64, cb_q, :],
                        start=True, stop=True)
                    p = ppool.tile([128, 512], F32R, tag="p", name=f"p{ln}")
                    nc.scalar.activation(p, st, EXP, bias=mb, scale=1.0)
                    nc.tensor.matmul(
                        oc, v_all[:, b * HPC + ln, kt, :], p,
                        start=(kt == 4 * j), stop=(kt == 4 * j + 3))
                ua = u_acc[(b, qb, ln)]
                if j == 0:
                    nc.vector.tensor_copy(ua, oc)
                else:
                    nc.vector.tensor_add(ua, ua, oc)

        def emit_norm(b, qb):
            for ln in range(HPC):
                ua = u_acc[(b, qb, ln)]
                rec = npool.tile([1, 512], F32, tag="rec", name=f"rec{ln}")
                nc.vector.reciprocal(rec, ua[64:65, :])
                recb = npool.tile([64, 512], F32, tag="recb", name=f"recb{ln}")
                nc.gpsimd.partition_broadcast(recb, rec)
                u16 = u16p.tile([64, 512], F16, tag="u16", name=f"u16_{qb}{ln}")
                u_16[(b, qb, ln)] = u16
                nc.vector.tensor_mul(u16, ua[0:64, :], recb)

        def emit_AG(b, bs):
            W = S * len(bs)
            cc_in = dram.tile([128, W], F16, tag="cc_in", name=f"ccin{b}")
            for bb in bs:
                for ln in range(HPC):
                    for qb in range(NQB):
                        nc.sync.dma_start(
                            out=cc_in[ln * 64:(ln + 1) * 64,
                                      (bb - bs[0]) * S + qb * 512:
                                      (bb - bs[0]) * S + (qb + 1) * 512],
                            in_=u_16[(bb, qb, ln)])
            if collectives:
                cc_out[b] = dram.tile([D, W], F16, tag="cc_out",
                                      name=f"ccout{b}", addr_space="Shared")
                nc.gpsimd.collective_compute(
                    "AllGather", mybir.AluOpType.bypass,
                    replica_groups=[list(range(NCORES))],
                    ins=[cc_in.opt()], outs=[cc_out[b].opt()])
            else:
                # sim-only stand-in: local copy with the same byte volume
                cc_out[b] = dram.tile([D, W], F16, tag="cc_out",
                                      name=f"ccout{b}")
                for r in range(NCORES):
                    nc.sync.dma_start(
                        out=cc_out[b][r * 128:(r + 1) * 128, :], in_=cc_in)

        def emit_wave(b):
            for c in range(NCBB):
                emit_B_cb(b, c)
                for j in range(c + 1):
                    emit_C_chunk(b, c, j)       # (qb=c, kt-chunk j)
                for q in range(c):
                    emit_C_chunk(b, q, c)       # (qb=q, kt-chunk c)
            for qb in range(NQB):
                emit_norm(b, qb)
            emit_AG(b, [b])

        def emit_D(b, hb):
            uqs = []
            for kt in range(DKT):
                uq = upool.tile([128, 1024], F16, tag="uq", name="uq")
                nc.sync.dma_start(
                    out=uq,
                    in_=cc_out[b][kt * 128:(kt + 1) * 128,
                                  hb * 1024:(hb + 1) * 1024])
                uqs.append(uq)
            for i in range(2):
                qb = hb * 2 + i
                fo = fo_ps.tile([128, 512], F32, tag="fo", name="fo")
                for kt in range(DKT):
                    nc.tensor.matmul(fo, w_sb["wo"][:, kt, :],
                                     uqs[kt][:, i * 512:(i + 1) * 512],
                                     start=(kt == 0), stop=(kt == DKT - 1))
                osb = opool.tile([128, 512], F32, tag="osb", name="osb")
                nc.vector.tensor_copy(osb, fo)
                nc.sync.dma_start(
                    out=out[:, b * S + qb * 512: b * S + (qb + 1) * 512],
                    in_=osb)

        with ExitStack() as bphase:
            xpool = bphase.enter_context(tc.tile_pool(name="xpool", bufs=2))
            cpool = bphase.enter_context(tc.tile_pool(name="cpool", bufs=2))
            tmp = bphase.enter_context(tc.tile_pool(name="tmp", bufs=3))
            vtp = bphase.enter_context(tc.tile_pool(name="vtp", bufs=1))
            proj_ps = bphase.enter_context(
                tc.tile_pool(name="proj_ps", bufs=2, space="PSUM"))
            vt = {b: vtp.tile([128, NCBB, 512], F32R, tag="vt", name=f"vt{b}")
                  for b in range(B)}

            emit_wave(0)
            nc.sync.dma_start(out=w_sb["wo"], in_=w_t["wo"])
            emit_wave(1)

            emit_D(0, 0)
            emit_D(0, 1)
            emit_D(1, 0)
            emit_D(1, 1)


# ---------------- host-side shard prep / unshard ----------------

def prep_inputs(x, attn_mask, Wq, Wk, Wv, Wo):
    """Full inputs -> list of 8 per-core input dicts."""
    x = np.asarray(x, dtype=np.float32)
    Wq = np.asarray(Wq, dtype=np.float32)
    Wk = np.asarray(Wk, dtype=np.float32)
    Wv = np.asarray(Wv, dtype=np.float32)
    Wo = np.asarray(Wo, dtype=np.float32)
    attn_mask = np.asarray(attn_mask)

    xT = np.ascontiguousarray(x.reshape(S2, D).T)          # [1024, 4096]

    # deinterleave: even hd components then odd, within each head
    comp = np.concatenate([np.arange(0, HD, 2), np.arange(1, HD, 2)])  # [64]
    half = HD // 2
    pi = np.concatenate([np.arange(half), np.arange(half)])            # [64]
    freq = np.float32(10000.0) ** (-2.0 * pi.astype(np.float32) / HD)
    pos = np.arange(S, dtype=np.float32)
    ang = pos[None, :] * freq[:, None]                     # [64, 2048]
    cos1 = np.cos(ang).astype(np.float32)
    sin1 = np.sin(ang).astype(np.float32)
    cosT = np.ascontiguousarray(
        np.tile(np.concatenate([cos1, cos1], axis=0), (1, B)))  # [128, 4096]
    sinT = np.ascontiguousarray(
        np.tile(np.concatenate([sin1, sin1], axis=0), (1, B)))

    permM = np.zeros((128, 128), dtype=np.float32)   # perm[p_in, p_out]
    for ln in range(HPC):
        base = ln * 64
        for j in range(half):
            permM[base + half + j, base + j] = -1.0
            permM[base + j, base + half + j] = 1.0
    identM = np.eye(128, dtype=np.float32)

    maskbM = np.zeros((128, B * NKT), dtype=np.float32)
    for b in range(B):
        for kt in range(NKT):
            mslice = attn_mask[b, kt * 128:(kt + 1) * 128]
            maskbM[:, b * NKT + kt] = np.where(
                mslice, np.float32(MASK_NEG), 0.0)

    in_maps = []
    for i in range(NCORES):
        heads = [HPC * i + ln for ln in range(HPC)]
        rows_qk = np.concatenate([h * HD + comp for h in heads])      # [128]
        rows_v = np.concatenate(
            [np.arange(h * HD, (h + 1) * HD) for h in heads])
        wq_i = np.ascontiguousarray((Wq[rows_qk, :] / 8.0).T)    # [1024, 128]
        wk_i = np.ascontiguousarray(Wk[rows_qk, :].T)
        wv_i = np.ascontiguousarray(Wv[rows_v, :].T)
        wo_i = np.ascontiguousarray(
            Wo[i * 128:(i + 1) * 128, :].T.astype(np.float16))
        in_maps.append({
            "xT": xT, "wq": wq_i, "wk": wk_i, "wv": wv_i, "wo": wo_i,
            "cosT": cosT, "sinT": sinT, "perm": permM, "ident": identM,
            "maskb": maskbM,
            "onesc": np.ones((128, B * HPC * NKT), dtype=np.float32),
        })
    return in_maps


def assemble_output(results):
    """list of per-core result dicts -> full [B, S, D] output."""
    cat = np.concatenate([results[i]["out"] for i in range(NCORES)], axis=0)
    # cat[n, b*S+s] -> out[b, s, n]
    return np.ascontiguousarray(cat.reshape(D, B, S).transpose(1, 2, 0))


_NC_CACHE = {}


def kernel(x, attn_mask, Wq, Wk, Wv, Wo):
    """Full-input, full-output entry point (shards across 8 NeuronCores)."""
    if "nc" not in _NC_CACHE:
        _NC_CACHE["nc"] = build_kernel()
    nc = _NC_CACHE["nc"]
    in_maps = prep_inputs(x, attn_mask, Wq, Wk, Wv, Wo)
    res = run_bass_kernel_spmd(nc, in_maps, core_ids=list(range(NCORES)))
    return assemble_output(res.results)



# revision 32
# speedup vs baseline: 1.4003x; 1.4003x over previous
"""Multi-head self-attention with RoPE — 8-core SPMD Bass kernel for TRN2 (v2).

Problem: nn_MultiHeadSelfAttention (b=2, s=2048, d=1024, h=16, hd=64),
y = softmax(mask(RoPE(xWq^T) RoPE(xWk^T)^T / 8)) (xWv^T) Wo^T.

Sharding (tensor/head parallel): heads 2i, 2i+1 -> core i. Each core
computes Q/K/V for its 2 heads over the full sequence, applies RoPE,
computes attention, and emits a row-sharded PARTIAL output projection
(Wo columns for its 128 context dims applied to all 1024 output dims):
no collectives — the host sums the 8 partial [1024, 4096] f16 outputs
(collectives measure ~335us/call in this environment vs ~10us tabled,
so the all-reduce is done during host-side unsharding instead).

v2 engineering notes (probe-driven, see session notes):
 - ALL matmuls are K=128 [128,128]x[128,512]. K=64 matmuls sliced at
   partition bases 0/64 (v1's per-head scores) measure 443ns vs 237ns,
   and alternating them with K=128 PV matmuls measures ~1777ns/pair on
   HW (cost model says 426ns). Scores instead use zero-padded per-head
   query operands: qz[head] has the other head's 64 partitions zeroed,
   so the key tile is a full [128,128] stationary operand shared by
   both heads. Probe: uniform-K128 st->exp->PV loop runs 474ns/iter vs
   1119ns for the K=64 version.
 - f16 activations end to end (fp32 PSUM accumulate): halves DMA and
   SBUF, 2x DVE, enables DMA-engine transposes. rel err ~1e-3 << 2e-2.
 - V is transposed to [keys, hd] via DMA-engine xbar transposes (f16),
   not PE transposes: no PSUM bank, no PE time, no DVE copies.
   V tiles are zero-padded per head like qz so head1's attention
   output lands on partitions 64:127 (out-proj rhs needs both heads
   stacked); the softmax denominators ride as a ones column per head
   (row 64 for head0, row 0 for head1).
 - Normalization: DVE reciprocal on the two denominator rows, one K=2
   PE matmul broadcasts them to [128,512], two aligned DVE muls produce
   the normalized f16 out-proj rhs. (v1's gpsimd partition_broadcast
   measured ~45us marginal for 16 calls.)
 - PSUM budget (8 banks): proj 2 + aux(perm/bc/fo shared) 1 + st 3 +
   oc0 1 + oc1 1 = 8. fo matmuls are emitted lagged one query-block so
   the shared aux bank's WAR waits are pre-satisfied.
"""

from contextlib import ExitStack

import numpy as np

import concourse.bacc as bacc_mod
import concourse.tile as tile
from concourse import mybir
from concourse.bass_utils import run_bass_kernel_spmd

F32 = mybir.dt.float32
F32R = mybir.dt.float32r
F16 = mybir.dt.float16
EXP = mybir.ActivationFunctionType.Exp

B = 2
S = 2048
D = 1024
H = 16
HD = 64
NCORES = 8
HPC = H // NCORES          # 2 heads per core
S2 = B * S                 # 4096
NKT = S // 128             # 16 key tiles per batch
NQB = S // 512             # 4 query blocks per batch
DKT = D // 128             # 8 contraction tiles for d=1024
MASK_NEG = -30.0
VW = 258                   # v_all cols: [h0 64 | one | Z64][one | Z63 | h1 64 | pad]


def build_kernel(repeats: int = 1, collectives: bool = True,
                 stages: frozenset = frozenset({"att", "norm", "outproj"})):
    nc = bacc_mod.Bacc("TRN2", target_bir_lowering=False, debug=False,
                       num_devices=NCORES)

    xT = nc.declare_dram_parameter("xT", [D, S2], F16, isOutput=False)
    wq = nc.declare_dram_parameter("wq", [D, 128], F16, isOutput=False)
    wk = nc.declare_dram_parameter("wk", [D, 128], F16, isOutput=False)
    wv = nc.declare_dram_parameter("wv", [D, 128], F16, isOutput=False)
    wo = nc.declare_dram_parameter("wo", [128, D], F16, isOutput=False)
    cosT = nc.declare_dram_parameter("cosT", [128, S2], F16, isOutput=False)
    sinT = nc.declare_dram_parameter("sinT", [128, S2], F16, isOutput=False)
    perm = nc.declare_dram_parameter("perm", [128, 128], F16, isOutput=False)
    sel = nc.declare_dram_parameter("sel", [2, 128], F32R, isOutput=False)
    maskb = nc.declare_dram_parameter("maskb", [128, B * NKT], F32,
                                      isOutput=False)
    onesc = nc.declare_dram_parameter("onesc", [128, 2 * B * NKT], F16,
                                      isOutput=False)
    out = nc.declare_dram_parameter("out", [D, S2], F16, isOutput=True)

    xT_t = xT.rearrange("(kt p) (b c) -> p kt b c", p=128, b=B)
    w_t = {n: w.rearrange("(kt p) m -> p kt m", p=128)
           for n, w in (("wq", wq), ("wk", wk), ("wv", wv))}
    wo_t = wo.rearrange("p (kt m) -> p kt m", m=128)
    cos_t = cosT.rearrange("p (b c) -> p b c", b=B)
    sin_t = sinT.rearrange("p (b c) -> p b c", b=B)
    out_t = out.rearrange("(kt p) c -> p kt c", p=128)

    with tile.TileContext(nc) as tc:
        with nc.allow_low_precision(
                reason="f16 activations; tolerance is 2e-2 rel"):
            for _ in range(repeats):
                _emit_body(nc, tc, xT_t, w_t, wo_t, cos_t, sin_t, perm, sel,
                           maskb, onesc, out_t, stages)
    nc.compile()
    return nc


def _emit_body(nc, tc, xT_t, w_t, wo_t, cos_t, sin_t, perm, sel, maskb, onesc,
               out_t, stages):
    with ExitStack() as body:
        consts = body.enter_context(tc.tile_pool(name="consts", bufs=1))
        w_sb = {}
        for n in ("wq", "wk", "wv"):
            w_sb[n] = consts.tile([128, DKT, 128], F16, name=f"{n}_sb")
            nc.sync.dma_start(out=w_sb[n], in_=w_t[n])
        w_sb["wo"] = consts.tile([128, DKT, 128], F16, name="wo_sb")
        nc.sync.dma_start(out=w_sb["wo"], in_=wo_t)
        perm_sb = consts.tile([128, 128], F16)
        nc.sync.dma_start(out=perm_sb, in_=perm[:, :])
        sel_sb = consts.tile([1, 2, 128], F32R)
        nc.sync.dma_start(out=sel_sb,
                          in_=sel.rearrange("(o t) m -> o t m", o=1))
        maskb_sb = consts.tile([128, B * NKT], F32)
        nc.sync.dma_start(out=maskb_sb, in_=maskb[:, :])

        # persistent activations
        acts = body.enter_context(tc.tile_pool(name="acts", bufs=1))
        krot = acts.tile([128, B, S], F16)
        v_all = acts.tile([128, B, NKT, VW], F16)
        # ones columns (denominator riders) + zero pad inside head1 tiles
        ones_r = onesc.rearrange("p (t b k) -> p t b k", t=2, b=B)
        nc.sync.dma_start(out=v_all[:, :, :, 64], in_=ones_r[:, 0])
        nc.sync.dma_start(out=v_all[:, :, :, 129], in_=ones_r[:, 1])
        nc.vector.memset(v_all[:, :, :, 130:193], 0.0)

        # PSUM: pr 2 + aux 1 + st 3 + oc0 1 + oc1 1 = 8 banks
        pr_ps = body.enter_context(
            tc.tile_pool(name="pr_ps", bufs=2, space="PSUM"))
        aux_ps = body.enter_context(
            tc.tile_pool(name="aux_ps", bufs=1, space="PSUM"))
        st_ps = body.enter_context(
            tc.tile_pool(name="st_ps", bufs=3, space="PSUM"))
        oc_ps = body.enter_context(
            tc.tile_pool(name="oc_ps", bufs=1, space="PSUM"))

        xpool = body.enter_context(tc.tile_pool(name="xpool", bufs=2))
        cpool = body.enter_context(tc.tile_pool(name="cpool", bufs=2))
        tmp = body.enter_context(tc.tile_pool(name="tmp", bufs=3))
        vtp = body.enter_context(tc.tile_pool(name="vtp", bufs=2))
        qzp = body.enter_context(tc.tile_pool(name="qzp", bufs=2))
        ppool = body.enter_context(tc.tile_pool(name="ppool", bufs=8))
        npool = body.enter_context(tc.tile_pool(name="npool", bufs=2))
        upool = body.enter_context(tc.tile_pool(name="upool", bufs=2))
        opool = body.enter_context(tc.tile_pool(name="opool", bufs=2))

        xsb = {}
        cos_sb = {}
        sin_sb = {}
        qz = {}
        oc = {}
        u_sb = {}

        def emit_xload(b):
            xsb[b] = xpool.tile([128, DKT, S], F16, tag="xsb",
                                name=f"xsb{b}")
            for half in range(2):
                nc.sync.dma_start(
                    out=xsb[b][:, 4 * half:4 * half + 4, :],
                    in_=xT_t[:, 4 * half:4 * half + 4, b, :])
            cos_sb[b] = cpool.tile([128, S], F16, tag="cos", name=f"cos{b}")
            sin_sb[b] = cpool.tile([128, S], F16, tag="sin", name=f"sin{b}")
            nc.sync.dma_start(out=cos_sb[b], in_=cos_t[:, b, :])
            nc.sync.dma_start(out=sin_sb[b], in_=sin_t[:, b, :])

        def emit_proj(b, c, name):
            """Project block c of batch b through w[name] -> f16 raw tile."""
            pr = pr_ps.tile([128, 512], F32, tag="pr", name="pr")
            for kt in range(DKT):
                nc.tensor.matmul(pr, w_sb[name][:, kt, :],
                                 xsb[b][:, kt, c * 512:(c + 1) * 512],
                                 start=(kt == 0), stop=(kt == DKT - 1))
            raw = tmp.tile([128, 512], F16, tag="raw", name="raw")
            nc.vector.tensor_copy(raw, pr)
            return raw

        def emit_rope(b, c, raw):
            """-> (tcos, tsin) f16 [128,512] to be added per-destination."""
            pp = aux_ps.tile([128, 512], F32, tag="aux", name="pp")
            nc.tensor.matmul(pp, perm_sb, raw, start=True, stop=True)
            tcos = tmp.tile([128, 512], F16, tag="tcos", name="tcos")
            nc.vector.tensor_mul(tcos, raw,
                                 cos_sb[b][:, c * 512:(c + 1) * 512])
            tsin = tmp.tile([128, 512], F16, tag="tsin", name="tsin")
            nc.vector.tensor_mul(tsin, pp,
                                 sin_sb[b][:, c * 512:(c + 1) * 512])
            return tcos, tsin

        def emit_KV(b, c):
            raw = emit_proj(b, c, "wk")
            tcos, tsin = emit_rope(b, c, raw)
            nc.vector.tensor_add(krot[:, b, c * 512:(c + 1) * 512],
                                 tcos, tsin)
            rawv = emit_proj(b, c, "wv")
            # one tiled DMA xbar transpose (key j -> tile j//128, partition
            # j%128), then aligned free-dim copies into the padded v layout
            vstage = vtp.tile([128, 4, 128], F16, tag="vstage",
                              name="vstage")
            nc.sync.dma_start(out=vstage, in_=rawv, transpose=True)
            for i in range(4):
                kt = 4 * c + i
                nc.vector.tensor_copy(v_all[:, b, kt, 0:64],
                                      vstage[:, i, 0:64])
                nc.vector.tensor_copy(v_all[:, b, kt, 193:257],
                                      vstage[:, i, 64:128])

        def emit_Q(b, qb):
            raw = emit_proj(b, qb, "wq")
            tcos, tsin = emit_rope(b, qb, raw)
            for ln in range(HPC):
                qzt = qzp.tile([128, 512], F16, tag=f"qz{ln}",
                               name=f"qz{ln}")
                qz[(b, qb, ln)] = qzt
                nc.vector.memset(qzt[(1 - ln) * 64:(2 - ln) * 64, :], 0.0)
                nc.vector.tensor_add(qzt[ln * 64:(ln + 1) * 64, :],
                                     tcos[ln * 64:(ln + 1) * 64, :],
                                     tsin[ln * 64:(ln + 1) * 64, :])

        def emit_st(b, qb, kt):
            """Score+exp for both heads of key tile kt -> (p0, p1)."""
            mb = maskb_sb[:, (b * NKT + kt):(b * NKT + kt) + 1]
            ps = []
            for ln in range(HPC):
                st = st_ps.tile([128, 512], F32, tag="st", name="st")
                nc.tensor.matmul(st, krot[:, b, kt * 128:(kt + 1) * 128],
                                 qz[(b, qb, ln)], start=True, stop=True)
                p = ppool.tile([128, 512], F16, tag="p", name="p")
                nc.scalar.activation(p, st, EXP, bias=mb, scale=1.0)
                ps.append(p)
            return ps

        def emit_oc(b, qb, kt, ps):
            for ln in range(HPC):
                if kt == 0:
                    shape = [65, 512] if ln == 0 else [128, 512]
                    oc[(b, qb, ln)] = oc_ps.tile(shape, F32, tag=f"oc{ln}",
                                                 name=f"oc{ln}")
                vsl = (v_all[:, b, kt, 0:65] if ln == 0
                       else v_all[:, b, kt, 129:257])
                nc.tensor.matmul(oc[(b, qb, ln)], vsl, ps[ln],
                                 start=(kt == 0), stop=(kt == NKT - 1))

        def emit_norm(b, qb):
            oc0 = oc[(b, qb, 0)]
            oc1 = oc[(b, qb, 1)]
            rec0 = npool.tile([1, 512], F32R, tag="rec0", name="rec0")
            nc.vector.reciprocal(rec0, oc0[64:65, :])
            rec1 = npool.tile([1, 512], F32R, tag="rec1", name="rec1")
            nc.vector.reciprocal(rec1, oc1[0:1, :])
            bc = aux_ps.tile([128, 512], F32, tag="aux", name="bc")
            nc.tensor.matmul(bc, sel_sb[:, 0, :], rec0, start=True,
                             stop=False)
            nc.tensor.matmul(bc, sel_sb[:, 1, :], rec1, start=False,
                             stop=True)
            bcs = npool.tile([128, 512], F32, tag="bcs", name="bcs")
            nc.vector.tensor_copy(bcs, bc)
            u = upool.tile([128, 512], F16, tag="u", name="u")
            u_sb[(b, qb)] = u
            import os
            if os.environ.get("SKIP_NORM"):
                nc.vector.tensor_copy(u[0:64, :], oc0[0:64, :])
                nc.vector.tensor_copy(u[64:128, :], oc1[64:128, :])
            else:
                nc.vector.tensor_mul(u[0:64, :], oc0[0:64, :], bcs[0:64, :])
                nc.vector.tensor_mul(u[64:128, :], oc1[64:128, :],
                                     bcs[64:128, :])

        osb8 = {}

        def emit_fo_one(b, qb, kt):
            if kt == 0:
                osb8[(b, qb)] = opool.tile([128, DKT, 512], F16, tag="osb",
                                           name="osb")
            fo = aux_ps.tile([128, 512], F32, tag="aux", name="fo")
            nc.tensor.matmul(fo, w_sb["wo"][:, kt, :], u_sb[(b, qb)],
                             start=True, stop=True)
            nc.vector.tensor_copy(osb8[(b, qb)][:, kt, :], fo)
            if kt == DKT - 1:
                nc.sync.dma_start(
                    out=out_t[:, :, b * S + qb * 512:b * S + (qb + 1) * 512],
                    in_=osb8[(b, qb)])

        def emit_att(b, qb, interleave):
            """16 kt chunks, software-pipelined (oc lags st/exp by one key
            tile so PV waits are pre-satisfied), with interleaved work items
            from `interleave`: a dict {kt: [callable, ...]}."""
            prev = None
            for kt in range(NKT):
                ps = emit_st(b, qb, kt)
                if prev is not None:
                    emit_oc(b, qb, kt - 1, prev)
                prev = ps
                for fn in interleave.get(kt, ()):
                    fn()
            emit_oc(b, qb, NKT - 1, prev)
            emit_norm(b, qb)

        def stub_out():
            osb0 = opool.tile([128, 512], F16, tag="stub", name="stub")
            src = u_sb[(0, 0)] if (0, 0) in u_sb else krot[:, 0, 0:512]
            nc.vector.tensor_copy(osb0, src)
            nc.sync.dma_start(out=out_t[:, 0, 0:512], in_=osb0)

        # ---------------- schedule ----------------
        emit_xload(0)
        for c in range(NQB):
            emit_KV(0, c)
        if "att" not in stages:
            emit_xload(1)
            for c in range(NQB):
                emit_KV(1, c)
            emit_Q(0, 0)
            emit_Q(1, 0)
            stub_out()
            return

        emit_Q(0, 0)
        pend = []           # deferred fo emissions (one per (b, qb))

        def make_fo_items(b, qb):
            return [lambda kt=kt: emit_fo_one(b, qb, kt)
                    for kt in range(DKT)]

        for b in range(B):
            for qb in range(NQB):
                inter = {}
                nxt = (b, qb + 1) if qb + 1 < NQB else (
                    (b + 1, 0) if b + 1 < B else None)
                if qb == NQB - 2 and b + 1 < B:
                    # start next batch's x/cos/sin DMAs one block early
                    inter[13] = [lambda bb=b + 1: emit_xload(bb)]
                if nxt is not None:
                    if nxt[1] == 0:
                        inter.setdefault(2, []).extend(
                            [lambda bb=nxt[0], cc=cc: emit_KV(bb, cc)
                             for cc in range(2)])
                        inter.setdefault(6, []).extend(
                            [lambda bb=nxt[0], cc=cc: emit_KV(bb, cc)
                             for cc in range(2, 4)])
                        inter.setdefault(10, []).append(
                            lambda bb=nxt[0]: emit_Q(bb, 0))
                    else:
                        inter[2] = [lambda bb=b, qq=qb + 1: emit_Q(bb, qq)]
                # spread pending fo matmuls through chunks 4..11
                if pend:
                    items = pend.pop(0)
                    for i, fn in enumerate(items):
                        inter.setdefault(4 + i, []).append(fn)
                emit_att(b, qb, inter)
                if "outproj" in stages:
                    pend.append(make_fo_items(b, qb))
        # drain remaining fo work (last query block)
        for items in pend:
            for fn in items:
                fn()
        if "outproj" not in stages:
            stub_out()


# ---------------- host-side shard prep / unshard ----------------

def prep_inputs(x, attn_mask, Wq, Wk, Wv, Wo):
    """Full inputs -> list of 8 per-core input dicts."""
    x = np.asarray(x, dtype=np.float32)
    Wq = np.asarray(Wq, dtype=np.float32)
    Wk = np.asarray(Wk, dtype=np.float32)
    Wv = np.asarray(Wv, dtype=np.float32)
    Wo = np.asarray(Wo, dtype=np.float32)
    attn_mask = np.asarray(attn_mask)

    xT = np.ascontiguousarray(x.reshape(S2, D).T.astype(np.float16))

    # deinterleave: even hd components then odd, within each head
    comp = np.concatenate([np.arange(0, HD, 2), np.arange(1, HD, 2)])  # [64]
    half = HD // 2
    pi = np.concatenate([np.arange(half), np.arange(half)])            # [64]
    freq = np.float32(10000.0) ** (-2.0 * pi.astype(np.float32) / HD)
    pos = np.arange(S, dtype=np.float32)
    ang = pos[None, :] * freq[:, None]                     # [64, 2048]
    cos1 = np.cos(ang).astype(np.float16)
    sin1 = np.sin(ang).astype(np.float16)
    cosT = np.ascontiguousarray(
        np.tile(np.concatenate([cos1, cos1], axis=0), (1, B)))  # [128, 4096]
    sinT = np.ascontiguousarray(
        np.tile(np.concatenate([sin1, sin1], axis=0), (1, B)))

    permM = np.zeros((128, 128), dtype=np.float16)   # perm[p_in, p_out]
    for ln in range(HPC):
        base = ln * 64
        for j in range(half):
            permM[base + half + j, base + j] = -1.0
            permM[base + j, base + half + j] = 1.0

    selM = np.zeros((2, 128), dtype=np.float32)
    selM[0, 0:64] = 1.0
    selM[1, 64:128] = 1.0

    maskbM = np.zeros((128, B * NKT), dtype=np.float32)
    for b in range(B):
        for kt in range(NKT):
            mslice = attn_mask[b, kt * 128:(kt + 1) * 128]
            maskbM[:, b * NKT + kt] = np.where(
                mslice, np.float32(MASK_NEG), 0.0)

    onescM = np.ones((128, 2 * B * NKT), dtype=np.float16)

    in_maps = []
    for i in range(NCORES):
        heads = [HPC * i + ln for ln in range(HPC)]
        rows_qk = np.concatenate([h * HD + comp for h in heads])      # [128]
        rows_v = np.concatenate(
            [np.arange(h * HD, (h + 1) * HD) for h in heads])
        wq_i = np.ascontiguousarray(
            (Wq[rows_qk, :] / 8.0).T.astype(np.float16))    # [1024, 128]
        wk_i = np.ascontiguousarray(Wk[rows_qk, :].T.astype(np.float16))
        wv_i = np.ascontiguousarray(Wv[rows_v, :].T.astype(np.float16))
        wo_i = np.ascontiguousarray(
            Wo[:, rows_v].T.astype(np.float16))             # [128, 1024]
        in_maps.append({
            "xT": xT, "wq": wq_i, "wk": wk_i, "wv": wv_i, "wo": wo_i,
            "cosT": cosT, "sinT": sinT, "perm": permM, "sel": selM,
            "maskb": maskbM, "onesc": onescM,
        })
    return in_maps


def assemble_output(results):
    """list of per-core result dicts -> full [B, S, D] output (sum of
    row-sharded partial projections)."""
    acc = np.zeros((D, S2), dtype=np.float32)
    for i in range(NCORES):
        acc += results[i]["out"].astype(np.float32)
    # acc[o, b*S+s] -> out[b, s, o]
    return np.ascontiguousarray(acc.reshape(D, B, S).transpose(1, 2, 0))


_NC_CACHE = {}


def kernel(x, attn_mask, Wq, Wk, Wv, Wo):
    """Full-input, full-output entry point (shards across 8 NeuronCores)."""
    if "nc" not in _NC_CACHE:
        _NC_CACHE["nc"] = build_kernel()
    nc = _NC_CACHE["nc"]
    in_maps = prep_inputs(x, attn_mask, Wq, Wk, Wv, Wo)
    res = run_bass_kernel_spmd(nc, in_maps, core_ids=list(range(NCORES)))
    return assemble_output(res.results)
